# revision 16
# baseline (speedup 1.0000x reference)
"""DecomposedSTFT power-spectrum kernel for 8 Trainium2 NeuronCores.

Reference computation:
    padded = reflect_pad(audio, 512)                       # [15361024]
    frame_t = padded[512*t : 512*t + 1024], t = 0..30000   # hop 512, win 1024
    ft[ch, t] = sum_n basis[ch, n] * frame_t[n]            # basis [1026, 1024]
    out[k, t] = ft[k, t]^2 + ft[513+k, t]^2                # k = 0..512

Structure exploited (fp16 storage, folded window, wide elementwise ops):
  * Window fold: basis[ch] = fb[ch] * win with periodic Hann win, so
    win[n+512] = 1 - win[n] and fb[ch, n+512] = (-1)^k fb[ch, n].  Hence
        ft[ch] = sum_{n<512} fb[ch, n] * (u[n] if k even else d[n])
        u_t[n] = win[n] x_t[n] + (1-win[n]) x_t[n+512]
        d_t[n] = win[n] x_t[n] - (1-win[n]) x_t[n+512]
    halving the matmul contraction from 1024 to 512.
  * imag rows 0 and 512 are identically zero, so the 1026 channels are
    exactly 1024 nonzero rows = 8 M-tiles of 128, grouped by bin parity:
    mt 0-3 read u (even bins), mt 4-7 read d (odd bins).  The Nyquist
    real row rides slot (mt=2, p=0); out rows 0/512 are DMA'd per chunk
    straight from the pair-0 square tile, and the pair-0 strip DMA
    skips partition 0, so no fixup ops are needed.
  * hop = win/2: audio laid out as 512-sample blocks (transposed host-side
    to [512, n_blocks]) makes x[n+512] a 1-column shift of x[n].
  * Elementwise op COUNT is minimized by operating on wide access
    patterns (HW-probed per-op costs: DVE stt 940-wide 1.09us — the f32
    scalar operand blocks the 16-bit fast mode; DVE tensor_add 1880-wide
    1.04us — 2x mode engages, strided sources free; ACT Square/scale
    940-wide 0.97us, including gap-APs across 2 PSUM banks; PE matmul
    470 cols 195.8ns with zero per-instruction overhead):
      - folds run once per 2-chunk window (940 frames): per kc one ACT
        z = win*x0 plus two independent DVE stts u = (1-win)*x1 + z and
        d = -(1-win)*x1 + z  (12 ops / 940 frames).
      - each power pair (ma, mb) accumulates into one 2-bank PSUM tile
        [128, 2, 512]; ONE 940-wide ACT Square covers both halves
        (4 squares/chunk instead of 8).
      - all four pairs square into one s tile [128, 4, 2, 470]; ONE
        1880-wide DVE add forms all four output row-blocks of the chunk
        (1 add/chunk instead of 4).
    Per iteration (8 chunks): PE 50.1us <- BINDING, ACT 46.3us, DVE
    43.2us, Pool ~10us (SWDGE only).  TimelineSim repeat-marginal sits
    at 50.08us = the PE roofline; sub-50 needs fewer matmul tiles, and
    FFT-style factorizations all cost more vector time than the PE time
    they save (a 128-row butterfly op ~0.5-1.1us on DVE vs 195.8ns per
    eliminated matmul; PE tile count ignores combine-matrix sparsity),
    fp8 fails the precision gate, and W = B*A factorizations need
    rank(W) <= 256 but W is full-rank.
  * software pipeline: x loads run ~1 window ahead, window folds issued
    after each chunk's squares so the in-order ACT queue never stalls the
    PSUM-recycling squares behind fold work.  No DMA rides the ACT- or
    DVE-driven DGE queues (a dma_start costs ~667ns on the issuing
    engine's sequencer, which would stall engine dispatch): input loads,
    the per-chunk [row0, Nyquist] pair, and the 256+2p / 257+2p strip row
    sets use the sync/HWDGE queue (SP seq is otherwise idle), and the
    0+2p / 1+2p row sets use the gpsimd/SWDGE queue (Pool DGE config is
    ~36ns).  Strips are batched per epoch ({0-3}, {4-6}, {7}) and
    deferred past each chunk's compute.

Sharding: frames split across 8 cores (3760 frames/core, surplus frames
cropped on gather); forward_basis replicated (weights derived exactly).
"""

import contextlib

import numpy as np

import concourse.bass as bass
import concourse.mybir as mybir
import concourse.tile as tile
from concourse import bacc
from concourse.bass_utils import run_bass_kernel_spmd

FL = 1024
HOP = 512
CUT = 513  # output rows
PAD = 512
L = 15360000
T_FRAMES = 30001

N_CORES = 8
F = 3760            # frames per core (8*3760 = 30080 >= 30001)
CHUNK = 470         # frames per N-chunk
N_CHUNKS = 8
GRP = 4             # chunks per output strip (out-DMA batching)
KC = 4              # K chunks of 128 (folded contraction dim 512)
MT = 8              # M tiles of 128 (1024 nonzero basis rows)

F16 = mybir.dt.float16
F32 = mybir.dt.float32

# power pairs (ma, mb) and the DRAM row offset of each pair's row set
# (row = off + 2p).  mt 0/1 = real even bins, 2/3 = [Nyq, imag even],
# 4/5 = real odd, 6/7 = imag odd.
PAIR_MTS = [(0, 2), (1, 3), (4, 6), (5, 7)]
ROW_OFFS = [0, 256, 1, 257]


def build_stft_nc(F=F, chunk=CHUNK, n_chunks=N_CHUNKS, repeat=1, warm_n=8,
                  unroll=False, no_out=0, no_loads=0):
    """Bass program: audio_t [512, F+1] x w [128, 4096] -> out [513, F] fp16.

    repeat > 1 wraps the compute in a hardware loop redoing identical work;
    used only for wall-clock benchmarking (unroll=True python-unrolls it
    instead, for TimelineSim).
    """
    assert chunk * n_chunks == F
    win2 = 2 * chunk            # frames per fold window
    assert n_chunks % 2 == 0
    nc = bacc.Bacc("TRN2", target_bir_lowering=False, debug=False)
    audio_t = nc.dram_tensor("audio_t", [512, F + 1], F16, kind="ExternalInput")
    w_dram = nc.dram_tensor("w", [128, KC * MT * 128], F16, kind="ExternalInput")
    winv_dram = nc.dram_tensor("winv", [128, 12], F32, kind="ExternalInput")
    out_dram = nc.dram_tensor("out", [CUT, F], F16, kind="ExternalOutput")

    with tile.TileContext(nc) as tc:
        with (
            tc.tile_pool(name="wpool", bufs=1) as wpool,
            tc.tile_pool(name="apool", bufs=3) as apool,
            tc.tile_pool(name="zpool", bufs=2) as zpool,
            tc.tile_pool(name="udpool", bufs=2) as udpool,
            tc.tile_pool(name="spool", bufs=3) as spool,
            tc.tile_pool(name="opool", bufs=2) as opool,
            tc.tile_pool(name="psum", bufs=1, space="PSUM") as pp,
        ):
            # head-latency critical path: interleave window-0 x loads with
            # the per-kc weight slices on one queue so the first matmul
            # group's weights and audio land together (transfers serialize
            # on the DMA engines, so ordering them x0,wt0,x1,wt1,... helps).
            # Window-0 x tiles get dedicated slots so repeat iterations can
            # re-fold them without the x-pool rotation clobbering them.
            winv = wpool.tile([128, 12], F32)
            nc.sync.dma_start(out=winv, in_=winv_dram[:, :])
            wt = wpool.tile([128, KC * MT * 128], F16)
            xfirst = {}
            for kc in range(KC):
                xf = wpool.tile([128, win2 + 1], F16, tag=f"xf{kc}",
                                name=f"xf{kc}")
                nc.sync.dma_start(
                    out=xf, in_=audio_t[kc * 128 : (kc + 1) * 128, 0 : win2 + 1]
                )
                xfirst[kc] = xf
                sl = slice(kc * MT * 128, (kc + 1) * MT * 128)
                nc.sync.dma_start(out=wt[:, sl], in_=w_dram[:, sl])
            xstatic = {0: xfirst}
            if no_loads:
                for w in range(1, n_chunks // 2):
                    xstatic[w] = {}
                    for kc in range(KC):
                        xf = wpool.tile([128, win2 + 1], F16,
                                        tag=f"xs{w}{kc}", name=f"xs{w}{kc}")
                        nc.sync.dma_start(
                            out=xf,
                            in_=audio_t[kc * 128 : (kc + 1) * 128,
                                        w * win2 : w * win2 + win2 + 1],
                        )
                        xstatic[w][kc] = xf

            # p-state warm-up bridge: the PE DVFS model runs 0.65/1.2GHz for
            # the first 3us of every continuous-execution run, reaching
            # 2.4GHz only after.  Real matmuls can't start until weights and
            # the first audio window land, so burn the ramp on dummy
            # matmuls that keep the PE busy from ~1us straight into the real
            # stream (any idle gap resets the ramp clock).
            if warm_n:
                warm = wpool.tile([2, 512], F16)
                nc.vector.memset(warm, 0.0)
                pwarm = pp.tile([128, 2, 512], F32, tag="q0")
                for _ in range(warm_n):
                    nc.tensor.matmul(
                        pwarm[0:2, 0, :], warm[:, 0:2], warm,
                        start=True, stop=True,
                    )

            loop_ctx = (
                tc.For_i(0, repeat, 1, hint_engines=(mybir.EngineType.PE,))
                if repeat > 1 and not unroll
                else contextlib.nullcontext()
            )
            n_unroll = repeat if (repeat > 1 and unroll) else 1

            # strip epochs: {0-3}, {4-6}, {7}: the last flush covers a single
            # chunk, so the end-of-kernel DMA transfers are small.
            epoch_of = {}
            e0 = 0
            while e0 < n_chunks:
                hi = min(e0 + GRP - 1, n_chunks - 1)
                if hi == n_chunks - 1 and hi > e0:
                    hi -= 1  # last chunk flushes alone
                for i in range(e0, hi + 1):
                    epoch_of[i] = (e0, hi)
                e0 = hi + 1

            n_win = n_chunks // 2

            def load_xw(w):
                """Audio block loads for fold window w (issued ~1 window
                ahead so epoch-boundary strip-out bursts never leave the
                window folds without input)."""
                xs = {}
                for kc in range(KC):
                    x = apool.tile([128, win2 + 1], F16, tag=f"x{kc}",
                                   name=f"x{kc}")
                    nc.sync.dma_start(
                        out=x,
                        in_=audio_t[
                            kc * 128 : (kc + 1) * 128,
                            w * win2 : w * win2 + win2 + 1,
                        ],
                    )
                    xs[kc] = x
                return xs

            def fold_part(xs, ud, kcs):
                """Window fold for a subset of kc: z = win*x0 on ACT, then
                two independent DVE stts u = (1-win)*x1 + z and
                d = -(1-win)*x1 + z.  940-wide ops, so fold cost is paid
                once per two chunks."""
                for kc in kcs:
                    x = xs[kc]
                    x1 = x[:, 1 : win2 + 1]
                    z = zpool.tile([128, win2], F16, tag=f"z{kc}")
                    nc.scalar.activation(
                        z, x[:, 0:win2], mybir.ActivationFunctionType.Copy,
                        scale=winv[:, kc : kc + 1],
                    )
                    u = udpool.tile([128, win2], F16, tag=f"u{kc}")
                    d = udpool.tile([128, win2], F16, tag=f"d{kc}")
                    nc.vector.scalar_tensor_tensor(
                        u, x1, winv[:, 4 + kc : 5 + kc], z,
                        mybir.AluOpType.mult, mybir.AluOpType.add,
                    )
                    nc.vector.scalar_tensor_tensor(
                        d, x1, winv[:, 8 + kc : 9 + kc], z,
                        mybir.AluOpType.mult, mybir.AluOpType.add,
                    )
                    ud[kc] = (u, d)

            with loop_ctx:
              for _rep in range(n_unroll):
                ostrip = None
                pending = []
                # software pipeline: window-0 fold + window-1 loads at body
                # top; thereafter loads run one window ahead and folds are
                # issued just after each chunk's squares, one window ahead
                # of their consuming matmuls.
                xq = {0: xfirst}
                if n_win > 1:
                    xq[1] = xstatic[1] if no_loads else load_xw(1)
                ud_w = {0: {}}
                fold_part(xq.pop(0), ud_w[0], range(KC))
                for n in range(n_chunks):
                    w, h = divmod(n, 2)
                    e0, e1 = epoch_of[n]
                    g = n - e0  # position within the output strip
                    if g == 0:
                        ostrip = {}
                        for pi in range(4):
                            ostrip[pi] = opool.tile(
                                [128, GRP * chunk], F16,
                                tag=f"o{pi}", name=f"ostrip{pi}",
                            )

                    c0 = n * chunk
                    ud = ud_w[w]
                    if h == 0 and w + 2 < n_win:
                        xq[w + 2] = (xstatic[w + 2] if no_loads
                                     else load_xw(w + 2))

                    cs = slice(h * chunk, (h + 1) * chunk)
                    s = spool.tile([128, 4, 2, chunk], F16, tag="s")
                    for pi, pair in enumerate(PAIR_MTS):
                        q = pp.tile([128, 2, 512], F32, tag=f"q{pi}")
                        for si, mt in enumerate(pair):
                            for kc in range(KC):
                                rhs = ud[kc][0] if mt < 4 else ud[kc][1]
                                nc.tensor.matmul(
                                    q[:, si, 0:chunk],
                                    wt[
                                        :,
                                        (kc * MT + mt) * 128
                                        : (kc * MT + mt + 1) * 128,
                                    ],
                                    rhs[:, cs],
                                    start=(kc == 0),
                                    stop=(kc == KC - 1),
                                )
                        # one 940-wide Square covers both matmul slots of
                        # the pair (gap AP across the 2 PSUM banks)
                        nc.scalar.activation(
                            s[:, pi, :, :], q[:, :, 0:chunk],
                            mybir.ActivationFunctionType.Square,
                        )
                    # per-pair adds into contiguous strip tiles (the 2x
                    # DVE mode engages; contiguous strips keep the out-DMA
                    # descriptors at 3760B/row — 940B scattered descriptors
                    # measured ~45us/iter slower on HW)
                    for pi in range(4):
                        nc.vector.tensor_add(
                            ostrip[pi][:, g * chunk : (g + 1) * chunk],
                            s[:, pi, 0, :], s[:, pi, 1, :],
                        )
                    # rows 0 and 512: bin-0 power is sq_mt0 row 0 alone
                    # (imag_0 == 0; the strip add's partition 0 is garbage
                    # for pair 0 and its strip DMA skips it), and Nyquist
                    # power is sq_mt2 row 0.  One tiny DMA per chunk on the
                    # sync queue covers both.
                    if not no_out:
                        nc.sync.dma_start(
                            out=bass.AP(
                                tensor=out_dram,
                                offset=c0,
                                ap=[[F, 1], [512 * F, 2], [1, chunk]],
                            ),
                            in_=s[0:1, 0, :, :],
                        )
                    if n == e1 and not no_out:
                        # strip complete: one DMA per pair row set, split
                        # over the gpsimd and sync DGE queues (never the
                        # ACT/DVE-driven queues); pair 0 skips partition 0
                        # (row 0 handled above).  Deferred past the chunk's
                        # compute so an in-order queue never blocks it.
                        nch = g + 1
                        for pi, row_off in enumerate(ROW_OFFS):
                            p0 = 1 if pi == 0 else 0
                            pending.append((
                                nc.gpsimd if pi in (0, 2) else nc.sync,
                                bass.AP(
                                    tensor=out_dram,
                                    offset=(row_off + 2 * p0) * F + e0 * chunk,
                                    ap=[[2 * F, 128 - p0], [1, nch * chunk]],
                                ),
                                ostrip[pi][p0:128, 0 : nch * chunk],
                            ))
                    # fold for the next window, issued after this chunk's
                    # squares so PSUM recycling is never stuck behind fold
                    # work in the in-order ACT queue
                    if w + 1 < n_win:
                        fold_next = ud_w.setdefault(w + 1, {})
                        fold_part(xq[w + 1], fold_next,
                                  (0, 1) if h == 0 else (2, 3))
                        if h == 1:
                            xq.pop(w + 1)
                            ud_w.pop(w, None)
                    for eng, oap, src in pending:
                        eng.dma_start(out=oap, in_=src)
                    pending.clear()
    nc.compile()
    return nc


def _win512():
    n = np.arange(512)
    return 0.5 * (1.0 - np.cos(2.0 * np.pi * n / FL))


def pack_weights(forward_basis):
    """[1026, 1, 1024] conv basis -> [128, 4096] folded lhsT tiles (fp16).

    fb[ch, n] = basis[ch, n] + (-1)^k basis[ch, n+512]  (exact unfold of the
    periodic-Hann window).  M layout groups bins by parity; imag rows 0/512
    (identically zero) are dropped and the Nyquist real row takes slot
    (mt=2, p=0).
    """
    basis = np.asarray(forward_basis, dtype=np.float64)[:, 0, :]  # [1026, 1024]
    k_of_ch = np.concatenate([np.arange(513), np.arange(513)])  # channel -> bin
    sign = np.where(k_of_ch % 2 == 0, 1.0, -1.0)
    fb = basis[:, :512] + sign[:, None] * basis[:, 512:]  # [1026, 512]

    ev = np.arange(0, 256, 2)
    od = np.arange(1, 256, 2)
    ch_tiles = [
        ev,                                     # mt0: real k = 0,2..254
        256 + ev,                               # mt1: real k = 256..510
        np.concatenate([[512], 513 + ev[1:]]),  # mt2: [real 512, imag 2..254]
        513 + 256 + ev,                         # mt3: imag k = 256..510
        od,                                     # mt4: real k = 1,3..255
        256 + od,                               # mt5: real k = 257..511
        513 + od,                               # mt6: imag k = 1..255
        513 + 256 + od,                         # mt7: imag k = 257..511
    ]
    w2 = np.empty((512, MT * 128), dtype=np.float64)  # [k, m]
    for mt, chs in enumerate(ch_tiles):
        assert len(chs) == 128, (mt, len(chs))
        w2[:, mt * 128 : (mt + 1) * 128] = fb[chs, :].T
    w_send = np.ascontiguousarray(
        w2.reshape(KC, 128, MT, 128).transpose(1, 0, 2, 3).reshape(128, -1)
    ).astype(np.float16)
    return w_send


def pack_winv():
    win = _win512()
    winv = np.empty((128, 12), dtype=np.float32)
    for kc in range(4):
        seg = win[kc * 128 : (kc + 1) * 128]
        winv[:, kc] = seg
        winv[:, 4 + kc] = 1.0 - seg
        winv[:, 8 + kc] = seg - 1.0
    return winv


def shard_audio(audio):
    """Full audio [15360000] -> per-core transposed fp16 blocks [512, F+1]."""
    padded = np.pad(np.asarray(audio, dtype=np.float32), PAD, mode="reflect")
    need = HOP * ((N_CORES - 1) * F + F + 1)  # samples covering all core spans
    ext = np.zeros(need, dtype=np.float32)
    ext[: padded.shape[0]] = padded
    shards = []
    for c in range(N_CORES):
        lo = HOP * c * F
        blk = ext[lo : lo + HOP * (F + 1)].reshape(F + 1, HOP)
        shards.append(np.ascontiguousarray(blk.T.astype(np.float16)))
    return shards


def kernel(audio, forward_basis):
    nc = build_stft_nc()
    w_send = pack_weights(forward_basis)
    winv = pack_winv()
    shards = shard_audio(audio)
    in_maps = [
        {"audio_t": shards[c], "w": w_send, "winv": winv} for c in range(N_CORES)
    ]
    res = run_bass_kernel_spmd(nc, in_maps, core_ids=list(range(N_CORES)))
    outs = [r["out"] for r in res.results]  # each [513, F] fp16
    full = np.concatenate(outs, axis=1)[:, :T_FRAMES]
    return full[None, :, :].astype(np.float32)


# revision 17
# speedup vs baseline: 1.1204x; 1.1204x over previous
"""DecomposedSTFT power-spectrum kernel for 8 Trainium2 NeuronCores.

Reference computation:
    padded = reflect_pad(audio, 512)                       # [15361024]
    frame_t = padded[512*t : 512*t + 1024], t = 0..30000   # hop 512, win 1024
    ft[ch, t] = sum_n basis[ch, n] * frame_t[n]            # basis [1026, 1024]
    out[k, t] = ft[k, t]^2 + ft[513+k, t]^2                # k = 0..512

Structure exploited (fp16 storage, folded window, wide elementwise ops):
  * Window fold: basis[ch] = fb[ch] * win with periodic Hann win, so
    win[n+512] = 1 - win[n] and fb[ch, n+512] = (-1)^k fb[ch, n].  Hence
        ft[ch] = sum_{n<512} fb[ch, n] * (u[n] if k even else d[n])
        u_t[n] = win[n] x_t[n] + (1-win[n]) x_t[n+512]
        d_t[n] = win[n] x_t[n] - (1-win[n]) x_t[n+512]
    halving the matmul contraction from 1024 to 512.
  * imag rows 0 and 512 are identically zero, so the 1026 channels are
    exactly 1024 nonzero rows = 8 M-tiles of 128, grouped by bin parity:
    mt 0-3 read u (even bins), mt 4-7 read d (odd bins).  The Nyquist
    real row rides slot (mt=2, p=0); out rows 0/512 are DMA'd per chunk
    straight from the pair-0 square tile, and the pair-0 strip DMA
    skips partition 0, so no fixup ops are needed.
  * hop = win/2: audio laid out as 512-sample blocks (transposed host-side
    to [512, n_blocks]) makes x[n+512] a 1-column shift of x[n].
  * Elementwise op COUNT is minimized by operating on wide access
    patterns (HW-probed per-op costs: DVE stt 940-wide 1.09us — the f32
    scalar operand blocks the 16-bit fast mode; DVE tensor_add 1880-wide
    1.04us — 2x mode engages, strided sources free; ACT Square/scale
    940-wide 0.97us, including gap-APs across 2 PSUM banks; PE matmul
    470 cols 195.8ns with zero per-instruction overhead):
      - folds run once per 2-chunk window (940 frames): per kc one ACT
        z = win*x0 plus two independent DVE stts u = (1-win)*x1 + z and
        d = -(1-win)*x1 + z  (12 ops / 940 frames).
      - each power pair (ma, mb) accumulates into one 2-bank PSUM tile
        [128, 2, 512]; ONE 940-wide ACT Square covers both halves
        (4 squares/chunk instead of 8).
      - all four pairs square into one s tile [128, 4, 2, 470]; ONE
        1880-wide DVE add forms all four output row-blocks of the chunk
        (1 add/chunk instead of 4).
    Per iteration (8 chunks): PE 50.1us <- BINDING, ACT 46.3us, DVE
    43.2us, Pool ~10us (SWDGE only).  TimelineSim repeat-marginal sits
    at 50.08us = the PE roofline; sub-50 needs fewer matmul tiles, and
    FFT-style factorizations all cost more vector time than the PE time
    they save (a 128-row butterfly op ~0.5-1.1us on DVE vs 195.8ns per
    eliminated matmul; PE tile count ignores combine-matrix sparsity),
    fp8 fails the precision gate, and W = B*A factorizations need
    rank(W) <= 256 but W is full-rank.
  * software pipeline: x loads run ~1 window ahead, window folds issued
    after each chunk's squares so the in-order ACT queue never stalls the
    PSUM-recycling squares behind fold work.  No DMA rides the ACT- or
    DVE-driven DGE queues (a dma_start costs ~667ns on the issuing
    engine's sequencer, which would stall engine dispatch): input loads,
    the per-chunk [row0, Nyquist] pair, and the 256+2p / 257+2p strip row
    sets use the sync/HWDGE queue (SP seq is otherwise idle), and the
    0+2p / 1+2p row sets use the gpsimd/SWDGE queue (Pool DGE config is
    ~36ns).  Strips are batched per epoch ({0-3}, {4-6}, {7}) and
    deferred past each chunk's compute.

Sharding: frames split across 8 cores (3760 frames/core, surplus frames
cropped on gather); forward_basis replicated (weights derived exactly).
"""

import contextlib

import numpy as np

import concourse.bass as bass
import concourse.mybir as mybir
import concourse.tile as tile
from concourse import bacc
from concourse.bass_utils import run_bass_kernel_spmd

FL = 1024
HOP = 512
CUT = 513  # output rows
PAD = 512
L = 15360000
T_FRAMES = 30001

N_CORES = 8
F = 3760            # frames per core (8*3760 = 30080 >= 30001)
CHUNK = 470         # frames per N-chunk
N_CHUNKS = 8
GRP = 4             # chunks per output strip (out-DMA batching)
KC = 4              # K chunks of 128 (folded contraction dim 512)
MT = 8              # M tiles of 128 (1024 nonzero basis rows)

F16 = mybir.dt.float16
F32 = mybir.dt.float32

# power pairs (ma, mb) and the DRAM row offset of each pair's row set
# (row = off + 2p).  mt 0/1 = real even bins, 2/3 = [Nyq, imag even],
# 4/5 = real odd, 6/7 = imag odd.
PAIR_MTS = [(0, 2), (1, 3), (4, 6), (5, 7)]
ROW_OFFS = [0, 256, 1, 257]


def build_stft_nc(F=F, chunk=CHUNK, n_chunks=N_CHUNKS, repeat=1, warm_n=8,
                  unroll=False, no_out=0, no_loads=0):
    """Bass program: audio_t [512, F+1] x w [128, 4096] -> out [513, F] fp16.

    repeat > 1 wraps the compute in a hardware loop redoing identical work;
    used only for wall-clock benchmarking (unroll=True python-unrolls it
    instead, for TimelineSim).
    """
    assert chunk * n_chunks == F
    win2 = 2 * chunk            # frames per fold window
    assert n_chunks % 2 == 0
    nc = bacc.Bacc("TRN2", target_bir_lowering=False, debug=False)
    audio_t = nc.dram_tensor("audio_t", [512, F + 1], F16, kind="ExternalInput")
    w_dram = nc.dram_tensor("w", [128, KC * MT * 128], F16, kind="ExternalInput")
    winv_dram = nc.dram_tensor("winv", [128, 12], F32, kind="ExternalInput")
    out_dram = nc.dram_tensor("out", [CUT, F], F16, kind="ExternalOutput")

    with tile.TileContext(nc) as tc:
        with (
            tc.tile_pool(name="wpool", bufs=1) as wpool,
            tc.tile_pool(name="apool", bufs=3) as apool,
            tc.tile_pool(name="zpool", bufs=2) as zpool,
            tc.tile_pool(name="udpool", bufs=2) as udpool,
            tc.tile_pool(name="spool", bufs=3) as spool,
            tc.tile_pool(name="opool", bufs=2) as opool,
            tc.tile_pool(name="psum", bufs=1, space="PSUM") as pp,
        ):
            # head-latency critical path: interleave window-0 x loads with
            # the per-kc weight slices on one queue so the first matmul
            # group's weights and audio land together (transfers serialize
            # on the DMA engines, so ordering them x0,wt0,x1,wt1,... helps).
            # Window-0 x tiles get dedicated slots so repeat iterations can
            # re-fold them without the x-pool rotation clobbering them.
            winv = wpool.tile([128, 12], F32)
            nc.sync.dma_start(out=winv, in_=winv_dram[:, :])
            wt = wpool.tile([128, KC * MT * 128], F16)
            xfirst = {}
            for kc in range(KC):
                xf = wpool.tile([128, win2 + 1], F16, tag=f"xf{kc}",
                                name=f"xf{kc}")
                nc.sync.dma_start(
                    out=xf, in_=audio_t[kc * 128 : (kc + 1) * 128, 0 : win2 + 1]
                )
                xfirst[kc] = xf
                sl = slice(kc * MT * 128, (kc + 1) * MT * 128)
                nc.sync.dma_start(out=wt[:, sl], in_=w_dram[:, sl])
            xstatic = {0: xfirst}
            if no_loads:
                for w in range(1, n_chunks // 2):
                    xstatic[w] = {}
                    for kc in range(KC):
                        xf = wpool.tile([128, win2 + 1], F16,
                                        tag=f"xs{w}{kc}", name=f"xs{w}{kc}")
                        nc.sync.dma_start(
                            out=xf,
                            in_=audio_t[kc * 128 : (kc + 1) * 128,
                                        w * win2 : w * win2 + win2 + 1],
                        )
                        xstatic[w][kc] = xf

            # p-state warm-up bridge: the PE DVFS model runs 0.65/1.2GHz for
            # the first 3us of every continuous-execution run, reaching
            # 2.4GHz only after.  Real matmuls can't start until weights and
            # the first audio window land, so burn the ramp on dummy
            # matmuls that keep the PE busy from ~1us straight into the real
            # stream (any idle gap resets the ramp clock).
            if warm_n:
                warm = wpool.tile([2, 512], F16)
                nc.vector.memset(warm, 0.0)
                pwarm = pp.tile([128, 2, 512], F32, tag="q0")
                for _ in range(warm_n):
                    nc.tensor.matmul(
                        pwarm[0:2, 0, :], warm[:, 0:2], warm,
                        start=True, stop=True,
                    )

            loop_ctx = (
                tc.For_i(0, repeat, 1, hint_engines=(mybir.EngineType.PE,))
                if repeat > 1 and not unroll
                else contextlib.nullcontext()
            )
            n_unroll = repeat if (repeat > 1 and unroll) else 1

            # strip epochs: {0-3}, {4-6}, {7}: the last flush covers a single
            # chunk, so the end-of-kernel DMA transfers are small.
            epoch_of = {}
            e0 = 0
            while e0 < n_chunks:
                hi = min(e0 + GRP - 1, n_chunks - 1)
                if hi == n_chunks - 1 and hi > e0:
                    hi -= 1  # last chunk flushes alone
                for i in range(e0, hi + 1):
                    epoch_of[i] = (e0, hi)
                e0 = hi + 1

            n_win = n_chunks // 2

            def load_xw(w):
                """Audio block loads for fold window w (issued ~1 window
                ahead so epoch-boundary strip-out bursts never leave the
                window folds without input)."""
                xs = {}
                for kc in range(KC):
                    x = apool.tile([128, win2 + 1], F16, tag=f"x{kc}",
                                   name=f"x{kc}")
                    nc.sync.dma_start(
                        out=x,
                        in_=audio_t[
                            kc * 128 : (kc + 1) * 128,
                            w * win2 : w * win2 + win2 + 1,
                        ],
                    )
                    xs[kc] = x
                return xs

            def fold_part(xs, ud, kcs):
                """Window fold for a subset of kc: z = win*x0 on ACT, then
                two independent DVE stts u = (1-win)*x1 + z and
                d = -(1-win)*x1 + z.  940-wide ops, so fold cost is paid
                once per two chunks."""
                for kc in kcs:
                    x = xs[kc]
                    x1 = x[:, 1 : win2 + 1]
                    z = zpool.tile([128, win2], F16, tag=f"z{kc}")
                    nc.scalar.activation(
                        z, x[:, 0:win2], mybir.ActivationFunctionType.Copy,
                        scale=winv[:, kc : kc + 1],
                    )
                    u = udpool.tile([128, win2], F16, tag=f"u{kc}")
                    d = udpool.tile([128, win2], F16, tag=f"d{kc}")
                    nc.vector.scalar_tensor_tensor(
                        u, x1, winv[:, 4 + kc : 5 + kc], z,
                        mybir.AluOpType.mult, mybir.AluOpType.add,
                    )
                    nc.vector.scalar_tensor_tensor(
                        d, x1, winv[:, 8 + kc : 9 + kc], z,
                        mybir.AluOpType.mult, mybir.AluOpType.add,
                    )
                    ud[kc] = (u, d)

            with loop_ctx:
              for _rep in range(n_unroll):
                ostrip = None
                pending = []
                # software pipeline: window-0 fold + window-1 loads at body
                # top; thereafter loads run one window ahead and folds are
                # issued just after each chunk's squares, one window ahead
                # of their consuming matmuls.
                xq = {0: xfirst}
                if n_win > 1:
                    xq[1] = xstatic[1] if no_loads else load_xw(1)
                ud_w = {0: {}}
                fold_part(xq.pop(0), ud_w[0], range(KC))
                for n in range(n_chunks):
                    w, h = divmod(n, 2)
                    e0, e1 = epoch_of[n]
                    g = n - e0  # position within the output strip
                    if g == 0:
                        ostrip = {}
                        for pi in range(4):
                            ostrip[pi] = opool.tile(
                                [128, GRP * chunk], F16,
                                tag=f"o{pi}", name=f"ostrip{pi}",
                            )

                    c0 = n * chunk
                    ud = ud_w[w]
                    if h == 0 and w + 2 < n_win:
                        xq[w + 2] = (xstatic[w + 2] if no_loads
                                     else load_xw(w + 2))

                    cs = slice(h * chunk, (h + 1) * chunk)
                    s = spool.tile([128, 4, 2, chunk], F16, tag="s")
                    for pi, pair in enumerate(PAIR_MTS):
                        q = pp.tile([128, 2, 512], F32, tag=f"q{pi}")
                        for si, mt in enumerate(pair):
                            for kc in range(KC):
                                rhs = ud[kc][0] if mt < 4 else ud[kc][1]
                                nc.tensor.matmul(
                                    q[:, si, 0:chunk],
                                    wt[
                                        :,
                                        (kc * MT + mt) * 128
                                        : (kc * MT + mt + 1) * 128,
                                    ],
                                    rhs[:, cs],
                                    start=(kc == 0),
                                    stop=(kc == KC - 1),
                                )
                        # one 940-wide Square covers both matmul slots of
                        # the pair (gap AP across the 2 PSUM banks)
                        nc.scalar.activation(
                            s[:, pi, :, :], q[:, :, 0:chunk],
                            mybir.ActivationFunctionType.Square,
                        )
                        if pi == 0 and not no_out:
                            # rows 0 and 512: bin-0 power is sq_mt0 row 0
                            # alone (imag_0 == 0; pair 0's strip add leaves
                            # garbage at partition 0 and its strip DMA skips
                            # it), Nyquist power is sq_mt2 row 0.  One tiny
                            # DMA per chunk, issued as soon as the pair-0
                            # square lands and on the gpsimd queue so its
                            # square-wait never blocks the sync queue's
                            # loads/strips.
                            nc.gpsimd.dma_start(
                                out=bass.AP(
                                    tensor=out_dram,
                                    offset=c0,
                                    ap=[[F, 1], [512 * F, 2], [1, chunk]],
                                ),
                                in_=s[0:1, 0, :, :],
                            )
                    # per-pair adds into contiguous strip tiles (the 2x
                    # DVE mode engages; contiguous strips keep the out-DMA
                    # descriptors at 3760B/row — 940B scattered descriptors
                    # measured ~45us/iter slower on HW)
                    for pi in range(4):
                        nc.vector.tensor_add(
                            ostrip[pi][:, g * chunk : (g + 1) * chunk],
                            s[:, pi, 0, :], s[:, pi, 1, :],
                        )
                    if n == e1 and not no_out:
                        # strip complete: one DMA per pair row set, split
                        # over the gpsimd and sync DGE queues (never the
                        # ACT/DVE-driven queues); pair 0 skips partition 0
                        # (row 0 handled above).  Deferred past the chunk's
                        # compute so an in-order queue never blocks it.
                        nch = g + 1
                        for pi, row_off in enumerate(ROW_OFFS):
                            p0 = 1 if pi == 0 else 0
                            pending.append((
                                nc.gpsimd if pi in (0, 2) else nc.sync,
                                bass.AP(
                                    tensor=out_dram,
                                    offset=(row_off + 2 * p0) * F + e0 * chunk,
                                    ap=[[2 * F, 128 - p0], [1, nch * chunk]],
                                ),
                                ostrip[pi][p0:128, 0 : nch * chunk],
                            ))
                    # fold for the next window, issued after this chunk's
                    # squares so PSUM recycling is never stuck behind fold
                    # work in the in-order ACT queue
                    if w + 1 < n_win:
                        fold_next = ud_w.setdefault(w + 1, {})
                        fold_part(xq[w + 1], fold_next,
                                  (0, 1) if h == 0 else (2, 3))
                        if h == 1:
                            xq.pop(w + 1)
                            ud_w.pop(w, None)
                    for eng, oap, src in pending:
                        eng.dma_start(out=oap, in_=src)
                    pending.clear()
    nc.compile()
    return nc


def _win512():
    n = np.arange(512)
    return 0.5 * (1.0 - np.cos(2.0 * np.pi * n / FL))


def pack_weights(forward_basis):
    """[1026, 1, 1024] conv basis -> [128, 4096] folded lhsT tiles (fp16).

    fb[ch, n] = basis[ch, n] + (-1)^k basis[ch, n+512]  (exact unfold of the
    periodic-Hann window).  M layout groups bins by parity; imag rows 0/512
    (identically zero) are dropped and the Nyquist real row takes slot
    (mt=2, p=0).
    """
    basis = np.asarray(forward_basis, dtype=np.float64)[:, 0, :]  # [1026, 1024]
    k_of_ch = np.concatenate([np.arange(513), np.arange(513)])  # channel -> bin
    sign = np.where(k_of_ch % 2 == 0, 1.0, -1.0)
    fb = basis[:, :512] + sign[:, None] * basis[:, 512:]  # [1026, 512]

    ev = np.arange(0, 256, 2)
    od = np.arange(1, 256, 2)
    ch_tiles = [
        ev,                                     # mt0: real k = 0,2..254
        256 + ev,                               # mt1: real k = 256..510
        np.concatenate([[512], 513 + ev[1:]]),  # mt2: [real 512, imag 2..254]
        513 + 256 + ev,                         # mt3: imag k = 256..510
        od,                                     # mt4: real k = 1,3..255
        256 + od,                               # mt5: real k = 257..511
        513 + od,                               # mt6: imag k = 1..255
        513 + 256 + od,                         # mt7: imag k = 257..511
    ]
    w2 = np.empty((512, MT * 128), dtype=np.float64)  # [k, m]
    for mt, chs in enumerate(ch_tiles):
        assert len(chs) == 128, (mt, len(chs))
        w2[:, mt * 128 : (mt + 1) * 128] = fb[chs, :].T
    w_send = np.ascontiguousarray(
        w2.reshape(KC, 128, MT, 128).transpose(1, 0, 2, 3).reshape(128, -1)
    ).astype(np.float16)
    return w_send


def pack_winv():
    win = _win512()
    winv = np.empty((128, 12), dtype=np.float32)
    for kc in range(4):
        seg = win[kc * 128 : (kc + 1) * 128]
        winv[:, kc] = seg
        winv[:, 4 + kc] = 1.0 - seg
        winv[:, 8 + kc] = seg - 1.0
    return winv


def shard_audio(audio):
    """Full audio [15360000] -> per-core transposed fp16 blocks [512, F+1]."""
    padded = np.pad(np.asarray(audio, dtype=np.float32), PAD, mode="reflect")
    need = HOP * ((N_CORES - 1) * F + F + 1)  # samples covering all core spans
    ext = np.zeros(need, dtype=np.float32)
    ext[: padded.shape[0]] = padded
    shards = []
    for c in range(N_CORES):
        lo = HOP * c * F
        blk = ext[lo : lo + HOP * (F + 1)].reshape(F + 1, HOP)
        shards.append(np.ascontiguousarray(blk.T.astype(np.float16)))
    return shards


def kernel(audio, forward_basis):
    nc = build_stft_nc()
    w_send = pack_weights(forward_basis)
    winv = pack_winv()
    shards = shard_audio(audio)
    in_maps = [
        {"audio_t": shards[c], "w": w_send, "winv": winv} for c in range(N_CORES)
    ]
    res = run_bass_kernel_spmd(nc, in_maps, core_ids=list(range(N_CORES)))
    outs = [r["out"] for r in res.results]  # each [513, F] fp16
    full = np.concatenate(outs, axis=1)[:, :T_FRAMES]
    return full[None, :, :].astype(np.float32)


# revision 18
# speedup vs baseline: 1.2729x; 1.1361x over previous
"""DecomposedSTFT power-spectrum kernel for 8 Trainium2 NeuronCores.

Reference computation:
    padded = reflect_pad(audio, 512)                       # [15361024]
    frame_t = padded[512*t : 512*t + 1024], t = 0..30000   # hop 512, win 1024
    ft[ch, t] = sum_n basis[ch, n] * frame_t[n]            # basis [1026, 1024]
    out[k, t] = ft[k, t]^2 + ft[513+k, t]^2                # k = 0..512

Structure exploited (fp16 storage, folded window, wide elementwise ops):
  * Window fold: basis[ch] = fb[ch] * win with periodic Hann win, so
    win[n+512] = 1 - win[n] and fb[ch, n+512] = (-1)^k fb[ch, n].  Hence
        ft[ch] = sum_{n<512} fb[ch, n] * (u[n] if k even else d[n])
        u_t[n] = win[n] x_t[n] + (1-win[n]) x_t[n+512]
        d_t[n] = win[n] x_t[n] - (1-win[n]) x_t[n+512]
    halving the matmul contraction from 1024 to 512.
  * imag rows 0 and 512 are identically zero, so the 1026 channels are
    exactly 1024 nonzero rows = 8 M-tiles of 128, grouped by bin parity:
    mt 0-3 read u (even bins), mt 4-7 read d (odd bins).  The Nyquist
    real row rides slot (mt=2, p=0); out rows 0/512 are DMA'd per chunk
    straight from the pair-0 square tile, and the pair-0 strip DMA
    skips partition 0, so no fixup ops are needed.
  * hop = win/2: audio laid out as 512-sample blocks (transposed host-side
    to [512, n_blocks]) makes x[n+512] a 1-column shift of x[n].
  * Elementwise op COUNT is minimized by operating on wide access
    patterns (HW-probed per-op costs: DVE stt 940-wide 1.09us — the f32
    scalar operand blocks the 16-bit fast mode; DVE tensor_add 1880-wide
    1.04us — 2x mode engages, strided sources free; ACT Square/scale
    940-wide 0.97us, including gap-APs across 2 PSUM banks; PE matmul
    470 cols 195.8ns with zero per-instruction overhead):
      - folds run once per 2-chunk window (940 frames): per kc one ACT
        z = win*x0 plus two independent DVE stts u = (1-win)*x1 + z and
        d = -(1-win)*x1 + z  (12 ops / 940 frames).
      - each power pair (ma, mb) accumulates into one 2-bank PSUM tile
        [128, 2, 512]; ONE 940-wide ACT Square covers both halves
        (4 squares/chunk instead of 8).
      - all four pairs square into one s tile [128, 4, 2, 470]; ONE
        1880-wide DVE add forms all four output row-blocks of the chunk
        (1 add/chunk instead of 4).
    Per iteration (8 chunks): PE 50.1us <- BINDING, ACT 46.3us, DVE
    43.2us, Pool ~10us (SWDGE only).  TimelineSim repeat-marginal sits
    at 50.08us = the PE roofline; sub-50 needs fewer matmul tiles, and
    FFT-style factorizations all cost more vector time than the PE time
    they save (a 128-row butterfly op ~0.5-1.1us on DVE vs 195.8ns per
    eliminated matmul; PE tile count ignores combine-matrix sparsity),
    fp8 fails the precision gate, and W = B*A factorizations need
    rank(W) <= 256 but W is full-rank.
  * software pipeline: x loads run ~1 window ahead, window folds issued
    after each chunk's squares so the in-order ACT queue never stalls the
    PSUM-recycling squares behind fold work.  No DMA rides the ACT- or
    DVE-driven DGE queues (a dma_start costs ~667ns on the issuing
    engine's sequencer, which would stall engine dispatch): input loads,
    the per-chunk [row0, Nyquist] pair, and the 256+2p / 257+2p strip row
    sets use the sync/HWDGE queue (SP seq is otherwise idle), and the
    0+2p / 1+2p row sets use the gpsimd/SWDGE queue (Pool DGE config is
    ~36ns).  Strips are batched per epoch ({0-3}, {4-6}, {7}) and
    deferred past each chunk's compute.

Sharding: frames split across 8 cores (3760 frames/core, surplus frames
cropped on gather); forward_basis replicated (weights derived exactly).
"""

import contextlib

import numpy as np

import concourse.bass as bass
import concourse.mybir as mybir
import concourse.tile as tile
from concourse import bacc
from concourse.bass_utils import run_bass_kernel_spmd

FL = 1024
HOP = 512
CUT = 513  # output rows
PAD = 512
L = 15360000
T_FRAMES = 30001

N_CORES = 8
F = 3752            # frames per core (8*3752 = 30016 >= 30001)
CHUNK = 469         # frames per N-chunk
N_CHUNKS = 8
GRP = 4             # chunks per output strip (out-DMA batching)
KC = 4              # K chunks of 128 (folded contraction dim 512)
MT = 8              # M tiles of 128 (1024 nonzero basis rows)

F16 = mybir.dt.float16
F32 = mybir.dt.float32

# power pairs (ma, mb) and the DRAM row offset of each pair's row set
# (row = off + 2p).  mt 0/1 = real even bins, 2/3 = [Nyq, imag even],
# 4/5 = real odd, 6/7 = imag odd.
PAIR_MTS = [(0, 2), (1, 3), (4, 6), (5, 7)]
ROW_OFFS = [0, 256, 1, 257]


def build_stft_nc(F=F, chunk=CHUNK, n_chunks=N_CHUNKS, repeat=1, warm_n=8,
                  unroll=False, no_out=0, no_loads=0):
    """Bass program: audio_t [512, F+1] x w [128, 4096] -> out [513, F] fp16.

    repeat > 1 wraps the compute in a hardware loop redoing identical work;
    used only for wall-clock benchmarking (unroll=True python-unrolls it
    instead, for TimelineSim).
    """
    assert chunk * n_chunks == F
    win2 = 2 * chunk            # frames per fold window
    assert n_chunks % 2 == 0
    nc = bacc.Bacc("TRN2", target_bir_lowering=False, debug=False)
    audio_t = nc.dram_tensor("audio_t", [512, F + 1], F16, kind="ExternalInput")
    w_dram = nc.dram_tensor("w", [128, KC * MT * 128], F16, kind="ExternalInput")
    winv_dram = nc.dram_tensor("winv", [128, 12], F32, kind="ExternalInput")
    out_dram = nc.dram_tensor("out", [CUT, F], F16, kind="ExternalOutput")

    with tile.TileContext(nc) as tc:
        with (
            tc.tile_pool(name="wpool", bufs=1) as wpool,
            tc.tile_pool(name="apool", bufs=3) as apool,
            tc.tile_pool(name="zpool", bufs=2) as zpool,
            tc.tile_pool(name="udpool", bufs=2) as udpool,
            tc.tile_pool(name="spool", bufs=3) as spool,
            tc.tile_pool(name="opool", bufs=2) as opool,
            tc.tile_pool(name="psum", bufs=1, space="PSUM") as pp,
        ):
            # head-latency critical path: interleave window-0 x loads with
            # the per-kc weight slices on one queue so the first matmul
            # group's weights and audio land together (transfers serialize
            # on the DMA engines, so ordering them x0,wt0,x1,wt1,... helps).
            # Window-0 x tiles get dedicated slots so repeat iterations can
            # re-fold them without the x-pool rotation clobbering them.
            winv = wpool.tile([128, 12], F32)
            nc.sync.dma_start(out=winv, in_=winv_dram[:, :])
            wt = wpool.tile([128, KC * MT * 128], F16)
            xfirst = {}
            for kc in range(KC):
                xf = wpool.tile([128, win2 + 1], F16, tag=f"xf{kc}",
                                name=f"xf{kc}")
                nc.sync.dma_start(
                    out=xf, in_=audio_t[kc * 128 : (kc + 1) * 128, 0 : win2 + 1]
                )
                xfirst[kc] = xf
                sl = slice(kc * MT * 128, (kc + 1) * MT * 128)
                nc.sync.dma_start(out=wt[:, sl], in_=w_dram[:, sl])
            xstatic = {0: xfirst}
            if no_loads:
                for w in range(1, n_chunks // 2):
                    xstatic[w] = {}
                    for kc in range(KC):
                        xf = wpool.tile([128, win2 + 1], F16,
                                        tag=f"xs{w}{kc}", name=f"xs{w}{kc}")
                        nc.sync.dma_start(
                            out=xf,
                            in_=audio_t[kc * 128 : (kc + 1) * 128,
                                        w * win2 : w * win2 + win2 + 1],
                        )
                        xstatic[w][kc] = xf

            # p-state warm-up bridge: the PE DVFS model runs 0.65/1.2GHz for
            # the first 3us of every continuous-execution run, reaching
            # 2.4GHz only after.  Real matmuls can't start until weights and
            # the first audio window land, so burn the ramp on dummy
            # matmuls that keep the PE busy from ~1us straight into the real
            # stream (any idle gap resets the ramp clock).
            if warm_n:
                warm = wpool.tile([2, 512], F16)
                nc.vector.memset(warm, 0.0)
                pwarm = pp.tile([128, 2, 512], F32, tag="q0")
                for _ in range(warm_n):
                    nc.tensor.matmul(
                        pwarm[0:2, 0, :], warm[:, 0:2], warm,
                        start=True, stop=True,
                    )

            loop_ctx = (
                tc.For_i(0, repeat, 1, hint_engines=(mybir.EngineType.PE,))
                if repeat > 1 and not unroll
                else contextlib.nullcontext()
            )
            n_unroll = repeat if (repeat > 1 and unroll) else 1

            # strip epochs: {0-3}, {4-6}, {7}: the last flush covers a single
            # chunk, so the end-of-kernel DMA transfers are small.
            epoch_of = {}
            e0 = 0
            while e0 < n_chunks:
                hi = min(e0 + GRP - 1, n_chunks - 1)
                if hi == n_chunks - 1 and hi > e0:
                    hi -= 1  # last chunk flushes alone
                for i in range(e0, hi + 1):
                    epoch_of[i] = (e0, hi)
                e0 = hi + 1

            n_win = n_chunks // 2

            def load_xw(w):
                """Audio block loads for fold window w (issued ~1 window
                ahead so epoch-boundary strip-out bursts never leave the
                window folds without input)."""
                xs = {}
                for kc in range(KC):
                    x = apool.tile([128, win2 + 1], F16, tag=f"x{kc}",
                                   name=f"x{kc}")
                    nc.sync.dma_start(
                        out=x,
                        in_=audio_t[
                            kc * 128 : (kc + 1) * 128,
                            w * win2 : w * win2 + win2 + 1,
                        ],
                    )
                    xs[kc] = x
                return xs

            def fold_part(xs, ud, kcs):
                """Window fold for a subset of kc: z = win*x0 on ACT, then
                two independent DVE stts u = (1-win)*x1 + z and
                d = -(1-win)*x1 + z.  940-wide ops, so fold cost is paid
                once per two chunks."""
                for kc in kcs:
                    x = xs[kc]
                    x1 = x[:, 1 : win2 + 1]
                    z = zpool.tile([128, win2], F16, tag=f"z{kc}")
                    nc.scalar.activation(
                        z, x[:, 0:win2], mybir.ActivationFunctionType.Copy,
                        scale=winv[:, kc : kc + 1],
                    )
                    u = udpool.tile([128, win2], F16, tag=f"u{kc}")
                    d = udpool.tile([128, win2], F16, tag=f"d{kc}")
                    nc.vector.scalar_tensor_tensor(
                        u, x1, winv[:, 4 + kc : 5 + kc], z,
                        mybir.AluOpType.mult, mybir.AluOpType.add,
                    )
                    nc.vector.scalar_tensor_tensor(
                        d, x1, winv[:, 8 + kc : 9 + kc], z,
                        mybir.AluOpType.mult, mybir.AluOpType.add,
                    )
                    ud[kc] = (u, d)

            with loop_ctx:
              for _rep in range(n_unroll):
                ostrip = None
                pending = []
                # software pipeline: window-0 fold + window-1 loads at body
                # top; thereafter loads run one window ahead and folds are
                # issued just after each chunk's squares, one window ahead
                # of their consuming matmuls.
                xq = {0: xfirst}
                if n_win > 1:
                    xq[1] = xstatic[1] if no_loads else load_xw(1)
                ud_w = {0: {}}
                fold_part(xq.pop(0), ud_w[0], range(KC))
                for n in range(n_chunks):
                    w, h = divmod(n, 2)
                    e0, e1 = epoch_of[n]
                    g = n - e0  # position within the output strip
                    if g == 0:
                        ostrip = {}
                        for pi in range(4):
                            ostrip[pi] = opool.tile(
                                [128, GRP * chunk], F16,
                                tag=f"o{pi}", name=f"ostrip{pi}",
                            )

                    c0 = n * chunk
                    ud = ud_w[w]
                    if h == 0 and w + 2 < n_win:
                        xq[w + 2] = (xstatic[w + 2] if no_loads
                                     else load_xw(w + 2))

                    cs = slice(h * chunk, (h + 1) * chunk)
                    s = spool.tile([128, 4, 2, chunk], F16, tag="s")
                    for pi, pair in enumerate(PAIR_MTS):
                        q = pp.tile([128, 2, 512], F32, tag=f"q{pi}")
                        for si, mt in enumerate(pair):
                            for kc in range(KC):
                                rhs = ud[kc][0] if mt < 4 else ud[kc][1]
                                nc.tensor.matmul(
                                    q[:, si, 0:chunk],
                                    wt[
                                        :,
                                        (kc * MT + mt) * 128
                                        : (kc * MT + mt + 1) * 128,
                                    ],
                                    rhs[:, cs],
                                    start=(kc == 0),
                                    stop=(kc == KC - 1),
                                )
                        # one 940-wide Square covers both matmul slots of
                        # the pair (gap AP across the 2 PSUM banks)
                        nc.scalar.activation(
                            s[:, pi, :, :], q[:, :, 0:chunk],
                            mybir.ActivationFunctionType.Square,
                        )
                        if pi == 0 and not no_out:
                            # rows 0 and 512: bin-0 power is sq_mt0 row 0
                            # alone (imag_0 == 0; pair 0's strip add leaves
                            # garbage at partition 0 and its strip DMA skips
                            # it), Nyquist power is sq_mt2 row 0.  One tiny
                            # DMA per chunk, issued as soon as the pair-0
                            # square lands and on the gpsimd queue so its
                            # square-wait never blocks the sync queue's
                            # loads/strips.
                            nc.gpsimd.dma_start(
                                out=bass.AP(
                                    tensor=out_dram,
                                    offset=c0,
                                    ap=[[F, 1], [512 * F, 2], [1, chunk]],
                                ),
                                in_=s[0:1, 0, :, :],
                            )
                    # per-pair adds into contiguous strip tiles (the 2x
                    # DVE mode engages; contiguous strips keep the out-DMA
                    # descriptors at 3760B/row — 940B scattered descriptors
                    # measured ~45us/iter slower on HW)
                    for pi in range(4):
                        nc.vector.tensor_add(
                            ostrip[pi][:, g * chunk : (g + 1) * chunk],
                            s[:, pi, 0, :], s[:, pi, 1, :],
                        )
                    if n == e1 and not no_out:
                        # strip complete: one DMA per pair row set, split
                        # over the gpsimd and sync DGE queues (never the
                        # ACT/DVE-driven queues); pair 0 skips partition 0
                        # (row 0 handled above).  Deferred past the chunk's
                        # compute so an in-order queue never blocks it.
                        nch = g + 1
                        for pi, row_off in enumerate(ROW_OFFS):
                            p0 = 1 if pi == 0 else 0
                            pending.append((
                                nc.gpsimd if pi in (0, 2) else nc.sync,
                                bass.AP(
                                    tensor=out_dram,
                                    offset=(row_off + 2 * p0) * F + e0 * chunk,
                                    ap=[[2 * F, 128 - p0], [1, nch * chunk]],
                                ),
                                ostrip[pi][p0:128, 0 : nch * chunk],
                            ))
                    # fold for the next window, issued after this chunk's
                    # squares so PSUM recycling is never stuck behind fold
                    # work in the in-order ACT queue
                    if w + 1 < n_win:
                        fold_next = ud_w.setdefault(w + 1, {})
                        fold_part(xq[w + 1], fold_next,
                                  (0, 1) if h == 0 else (2, 3))
                        if h == 1:
                            xq.pop(w + 1)
                            ud_w.pop(w, None)
                    for eng, oap, src in pending:
                        eng.dma_start(out=oap, in_=src)
                    pending.clear()
    nc.compile()
    return nc


def _win512():
    n = np.arange(512)
    return 0.5 * (1.0 - np.cos(2.0 * np.pi * n / FL))


def pack_weights(forward_basis):
    """[1026, 1, 1024] conv basis -> [128, 4096] folded lhsT tiles (fp16).

    fb[ch, n] = basis[ch, n] + (-1)^k basis[ch, n+512]  (exact unfold of the
    periodic-Hann window).  M layout groups bins by parity; imag rows 0/512
    (identically zero) are dropped and the Nyquist real row takes slot
    (mt=2, p=0).
    """
    basis = np.asarray(forward_basis, dtype=np.float64)[:, 0, :]  # [1026, 1024]
    k_of_ch = np.concatenate([np.arange(513), np.arange(513)])  # channel -> bin
    sign = np.where(k_of_ch % 2 == 0, 1.0, -1.0)
    fb = basis[:, :512] + sign[:, None] * basis[:, 512:]  # [1026, 512]

    ev = np.arange(0, 256, 2)
    od = np.arange(1, 256, 2)
    ch_tiles = [
        ev,                                     # mt0: real k = 0,2..254
        256 + ev,                               # mt1: real k = 256..510
        np.concatenate([[512], 513 + ev[1:]]),  # mt2: [real 512, imag 2..254]
        513 + 256 + ev,                         # mt3: imag k = 256..510
        od,                                     # mt4: real k = 1,3..255
        256 + od,                               # mt5: real k = 257..511
        513 + od,                               # mt6: imag k = 1..255
        513 + 256 + od,                         # mt7: imag k = 257..511
    ]
    w2 = np.empty((512, MT * 128), dtype=np.float64)  # [k, m]
    for mt, chs in enumerate(ch_tiles):
        assert len(chs) == 128, (mt, len(chs))
        w2[:, mt * 128 : (mt + 1) * 128] = fb[chs, :].T
    w_send = np.ascontiguousarray(
        w2.reshape(KC, 128, MT, 128).transpose(1, 0, 2, 3).reshape(128, -1)
    ).astype(np.float16)
    return w_send


def pack_winv():
    win = _win512()
    winv = np.empty((128, 12), dtype=np.float32)
    for kc in range(4):
        seg = win[kc * 128 : (kc + 1) * 128]
        winv[:, kc] = seg
        winv[:, 4 + kc] = 1.0 - seg
        winv[:, 8 + kc] = seg - 1.0
    return winv


def shard_audio(audio):
    """Full audio [15360000] -> per-core transposed fp16 blocks [512, F+1]."""
    padded = np.pad(np.asarray(audio, dtype=np.float32), PAD, mode="reflect")
    need = HOP * ((N_CORES - 1) * F + F + 1)  # samples covering all core spans
    ext = np.zeros(need, dtype=np.float32)
    ext[: padded.shape[0]] = padded
    shards = []
    for c in range(N_CORES):
        lo = HOP * c * F
        blk = ext[lo : lo + HOP * (F + 1)].reshape(F + 1, HOP)
        shards.append(np.ascontiguousarray(blk.T.astype(np.float16)))
    return shards


def kernel(audio, forward_basis):
    nc = build_stft_nc()
    w_send = pack_weights(forward_basis)
    winv = pack_winv()
    shards = shard_audio(audio)
    in_maps = [
        {"audio_t": shards[c], "w": w_send, "winv": winv} for c in range(N_CORES)
    ]
    res = run_bass_kernel_spmd(nc, in_maps, core_ids=list(range(N_CORES)))
    outs = [r["out"] for r in res.results]  # each [513, F] fp16
    full = np.concatenate(outs, axis=1)[:, :T_FRAMES]
    return full[None, :, :].astype(np.float32)
